# revision 7
# baseline (speedup 1.0000x reference)
"""Trainium2 Bass kernel for CompactnessLoss (segment-reduce over K=64 clusters).

loss = sum_{k: n_k>1} [ sum_{i in k} ||x_i||^2 - ||s_k||^2 / n_k ],   s_k = sum_{i in k} x_i

Identity used on device (avoids needing per-cluster sum-of-squares):
    loss = T1 - sum_k normsq_k * ( 1[n_k>1]/n_k + 1[n_k==1] )
where T1 = sum_i ||x_i||^2 over ALL rows (clusters with n_k==1 contribute
normsq_k == their single row's squared norm, n_k==0 contribute nothing).

Strategy (8 NeuronCores, data-parallel over N):
  - Shard N=200000 rows -> 25000/core, pad to 25088 = 7 chunks x 128 part x 28 rows.
  - Host packs features as bf16 [25088, 257] (col 256 = 1.0 ones column for counts;
    padding rows are all-zero with assignment=64 so they match no cluster).
  - Per chunk: one ~1.85MB DMA; DVE builds one-hot [128,28,64] via broadcast
    is_equal(assign, iota); PE accumulates onehot^T @ [x | 1] into PSUM [64,257]
    (cols 0..255 = per-cluster sums, col 256 = per-cluster counts); ACT or DVE
    computes sum(x^2) per partition-chunk (split across both engines to balance).
  - AllReduce the [64,258] partial (sums, counts, T1) across the 8 cores, finish
    the tiny scalar math identically on each core, output one f32 scalar.
"""

import numpy as np
import ml_dtypes

import concourse.bacc as bacc
import concourse.bass as bass
import concourse.tile as tile
from concourse import mybir
from concourse.bass_utils import run_bass_kernel_spmd

BF16 = mybir.dt.bfloat16
F32 = mybir.dt.float32
P = 128
K = 64            # num clusters
D = 256           # feature dim
MOV = D + 1       # moving columns: features + ones

# full-size problem config
N_TOTAL = 200000
N_CORES = 8
ROWS_REAL = N_TOTAL // N_CORES      # 25000
CHUNK = 28                          # subtiles (matmuls) per DMA chunk
N_CHUNKS = 7
ROWS_PAD = N_CHUNKS * CHUNK * P     # 25088
# chunks whose square+rowsum runs on ScalarE (rest on VectorE): balance engines
ACT_CHUNKS = frozenset({0, 2, 3, 5})


def build_nc(n_cores=N_CORES, n_chunks=N_CHUNKS, chunk=CHUNK,
             act_chunks=ACT_CHUNKS, correction=float(N_TOTAL),
             debug_partial=False, use_ttr=False):
    """Build the SPMD Bass program. Inputs per core:
       feat [rows_pad, MOV] bf16, assign_t [P, n_sub] bf16, iota [P, K] bf16.
       Output: out [1,1] f32 (identical on every core)."""
    n_sub = n_chunks * chunk
    rows_pad = n_sub * P
    block = chunk * P  # rows per chunk

    nc = bacc.Bacc("TRN2", target_bir_lowering=False, debug=False,
                   num_devices=n_cores)

    feat_d = nc.dram_tensor("feat", [rows_pad, MOV], BF16, kind="ExternalInput")
    assign_d = nc.dram_tensor("assign_t", [P, n_sub], BF16, kind="ExternalInput")
    iota_d = nc.dram_tensor("iota", [P, K], BF16, kind="ExternalInput")
    if debug_partial:
        out_d = nc.dram_tensor("out", [K, MOV + 1], F32, kind="ExternalOutput")
    else:
        out_d = nc.dram_tensor("out", [1, 1], F32, kind="ExternalOutput")

    with tile.TileContext(nc) as tc:
        with (
            tc.tile_pool(name="io", bufs=1) as io,
            tc.tile_pool(name="bufp", bufs=3) as bufp,
            tc.tile_pool(name="ohp", bufs=3) as ohp,
            tc.tile_pool(name="psum", bufs=1, space="PSUM") as psum,
            tc.tile_pool(name="dram", bufs=1, space="DRAM") as dram,
        ):
            asg = io.tile([P, n_sub], BF16)
            nc.sync.dma_start(out=asg[:], in_=assign_d[:])
            iot = io.tile([P, K], BF16)
            nc.sync.dma_start(out=iot[:], in_=iota_d[:])

            ones_sb = io.tile([P, 1], F32)
            nc.vector.memset(ones_sb[:], 1.0)
            t1a = io.tile([P, n_chunks], F32)
            nc.vector.memset(t1a[:], 0.0)
            t1d = io.tile([P, n_chunks], F32)
            nc.vector.memset(t1d[:], 0.0)
            scr_a = io.tile([P, chunk, MOV], BF16)
            scr_d = io.tile([P, chunk, MOV], BF16)

            acc = psum.tile([K, MOV], F32, space="PSUM")
            feat_ap = feat_d[:]

            for s in range(n_chunks):
                buf = bufp.tile([P, chunk, MOV], BF16, name="buf")
                nc.sync.dma_start(
                    out=buf[:],
                    in_=feat_ap[s * block:(s + 1) * block, :].rearrange(
                        "(p n) m -> p n m", n=chunk),
                )
                oh = ohp.tile([P, chunk, K], BF16, name="oh")
                nc.vector.tensor_tensor(
                    out=oh[:],
                    in0=asg[:, s * chunk:(s + 1) * chunk]
                        .unsqueeze(-1).to_broadcast([P, chunk, K]),
                    in1=iot[:].unsqueeze(1).to_broadcast([P, chunk, K]),
                    op=mybir.AluOpType.is_equal,
                )
                if s in act_chunks:
                    nc.scalar.activation(
                        out=scr_a[:], in_=buf[:],
                        func=mybir.ActivationFunctionType.Square,
                        accum_out=t1a[:, s:s + 1],
                    )
                elif use_ttr:
                    nc.vector.tensor_tensor_reduce(
                        out=scr_d[:], in0=buf[:], in1=buf[:],
                        scale=1.0, scalar=0.0,
                        op0=mybir.AluOpType.mult, op1=mybir.AluOpType.add,
                        accum_out=t1d[:, s:s + 1],
                    )
                else:
                    nc.vector.tensor_tensor(
                        out=scr_d[:], in0=buf[:], in1=buf[:],
                        op=mybir.AluOpType.mult)
                    nc.vector.reduce_sum(
                        out=t1d[:, s:s + 1], in_=scr_d[:],
                        axis=mybir.AxisListType.XY)
                for j in range(chunk):
                    nc.tensor.matmul(
                        out=acc[:], lhsT=oh[:, j, :], rhs=buf[:, j, :],
                        start=(s == 0 and j == 0),
                        stop=(s == n_chunks - 1 and j == chunk - 1),
                    )

            # per-core T1 partial: reduce [P, n_chunks] cols, then partitions via PE
            t1vec = io.tile([P, 1], F32)
            nc.vector.tensor_tensor(out=t1a[:], in0=t1a[:], in1=t1d[:],
                                    op=mybir.AluOpType.add)
            nc.vector.reduce_sum(out=t1vec[:], in_=t1a[:],
                                 axis=mybir.AxisListType.X)
            t1p = psum.tile([1, 1], F32, space="PSUM")
            nc.tensor.matmul(out=t1p[:], lhsT=t1vec[:], rhs=ones_sb[:],
                             start=True, stop=True)

            # pack [64, 258]: cols 0..255 sums, 256 counts, 257 T1 (row 0 only)
            partial = io.tile([K, MOV + 1], F32)
            nc.vector.memset(partial[:], 0.0)
            nc.scalar.copy(out=partial[:, 0:MOV], in_=acc[:])
            nc.scalar.copy(out=partial[0:1, MOV:MOV + 1], in_=t1p[:])

            if debug_partial:
                nc.sync.dma_start(out=out_d[:], in_=partial[:])
            else:
                ar_in = dram.tile([K, MOV + 1], F32)
                ar_out = dram.tile([K, MOV + 1], F32)
                nc.sync.dma_start(out=ar_in[:], in_=partial[:])
                nc.gpsimd.collective_compute(
                    "AllReduce", mybir.AluOpType.add,
                    replica_groups=[list(range(n_cores))],
                    ins=[ar_in[:].opt()], outs=[ar_out[:].opt()],
                )
                red = io.tile([K, MOV + 1], F32)
                nc.sync.dma_start(out=red[:], in_=ar_out[:])

                # finisher (identical on all cores)
                scr_f = io.tile([K, D], F32)
                normsq = io.tile([K, 1], F32)
                if use_ttr:
                    nc.vector.tensor_tensor_reduce(
                        out=scr_f[:], in0=red[:, 0:D], in1=red[:, 0:D],
                        scale=1.0, scalar=0.0,
                        op0=mybir.AluOpType.mult, op1=mybir.AluOpType.add,
                        accum_out=normsq[:],
                    )
                else:
                    nc.vector.tensor_tensor(
                        out=scr_f[:], in0=red[:, 0:D], in1=red[:, 0:D],
                        op=mybir.AluOpType.mult)
                    nc.vector.reduce_sum(out=normsq[:], in_=scr_f[:],
                                         axis=mybir.AxisListType.X)
                counts = red[:, D:D + 1]
                safe = io.tile([K, 1], F32)
                nc.vector.tensor_scalar(out=safe[:], in0=counts, scalar1=1.0,
                                        scalar2=None, op0=mybir.AluOpType.max)
                inv = io.tile([K, 1], F32)
                nc.vector.reciprocal(out=inv[:], in_=safe[:])
                maskgt = io.tile([K, 1], F32)
                nc.vector.tensor_scalar(out=maskgt[:], in0=counts, scalar1=1.0,
                                        scalar2=None,
                                        op0=mybir.AluOpType.is_gt)
                maskeq = io.tile([K, 1], F32)
                nc.vector.tensor_scalar(out=maskeq[:], in0=counts, scalar1=1.0,
                                        scalar2=None,
                                        op0=mybir.AluOpType.is_equal)
                w = io.tile([K, 1], F32)
                nc.vector.tensor_tensor(out=w[:], in0=maskgt[:], in1=inv[:],
                                        op=mybir.AluOpType.mult)
                nc.vector.tensor_tensor(out=w[:], in0=w[:], in1=maskeq[:],
                                        op=mybir.AluOpType.add)
                sub = io.tile([K, 1], F32)
                nc.vector.tensor_tensor(out=sub[:], in0=normsq[:], in1=w[:],
                                        op=mybir.AluOpType.mult)
                subp = psum.tile([1, 1], F32, space="PSUM")
                nc.tensor.matmul(out=subp[:], lhsT=sub[:],
                                 rhs=ones_sb[0:K, :], start=True, stop=True)

                tmp = io.tile([1, 1], F32)
                nc.vector.tensor_scalar(out=tmp[:], in0=red[0:1, MOV:MOV + 1],
                                        scalar1=correction, scalar2=None,
                                        op0=mybir.AluOpType.subtract)
                loss_sb = io.tile([1, 1], F32)
                nc.vector.tensor_tensor(out=loss_sb[:], in0=tmp[:],
                                        in1=subp[0:1, 0:1],
                                        op=mybir.AluOpType.subtract)
                nc.sync.dma_start(out=out_d[:], in_=loss_sb[:])

    nc.compile()
    return nc


def prep_inputs(features, cluster_assignments, n_cores=N_CORES,
                n_chunks=N_CHUNKS, chunk=CHUNK):
    """Shard + pack host inputs. Returns in_maps for run_bass_kernel_spmd."""
    bf = ml_dtypes.bfloat16
    n_sub = n_chunks * chunk
    rows_pad = n_sub * P
    block = chunk * P
    n_total = features.shape[0]
    rows_real = n_total // n_cores
    assert rows_real * n_cores == n_total

    feats = np.asarray(features, dtype=np.float32)
    asg = np.asarray(cluster_assignments).astype(np.float32)

    iota = np.broadcast_to(np.arange(K, dtype=np.float32), (P, K)).astype(bf)

    in_maps = []
    for c in range(n_cores):
        fpad = np.zeros((rows_pad, MOV), dtype=np.float32)
        fpad[:rows_real, :D] = feats[c * rows_real:(c + 1) * rows_real]
        fpad[:rows_real, D] = 1.0
        apad = np.full((rows_pad,), float(K), dtype=np.float32)
        apad[:rows_real] = asg[c * rows_real:(c + 1) * rows_real]
        # assign_t[p, s*chunk + j] must be the cluster of feat row s*block + p*chunk + j
        assign_t = (apad.reshape(n_chunks, P, chunk)
                    .transpose(1, 0, 2).reshape(P, n_sub))
        in_maps.append({
            "feat": fpad.astype(bf),
            "assign_t": assign_t.astype(bf),
            "iota": iota,
        })
    return in_maps


_NC_CACHE = {}


def kernel(features, cluster_assignments):
    key = "full"
    if key not in _NC_CACHE:
        _NC_CACHE[key] = build_nc()
    nc = _NC_CACHE[key]
    in_maps = prep_inputs(features, cluster_assignments)
    res = run_bass_kernel_spmd(nc, in_maps, core_ids=list(range(N_CORES)))
    loss = res.results[0]["out"]
    return np.float32(loss.reshape(())).reshape(())


if __name__ == "__main__":
    rng = np.random.default_rng(0)
    f = rng.standard_normal((N_TOTAL, D)).astype(np.float32)
    a = rng.integers(0, K, size=(N_TOTAL,)).astype(np.int64)
    got = kernel(f, a)
    # numpy reference
    oh = np.zeros((N_TOTAL, K), np.float32)
    oh[np.arange(N_TOTAL), a] = 1.0
    counts = oh.sum(0)
    sums = oh.T @ f
    sumsq = oh.T @ (f * f).sum(1)
    per = sumsq - (sums * sums).sum(1) / np.maximum(counts, 1.0)
    want = per[counts > 1].sum()
    print("got", got, "want", want, "rel", abs(got - want) / abs(want))


# revision 10
# speedup vs baseline: 1.1323x; 1.1323x over previous
"""Trainium2 Bass kernel for CompactnessLoss (segment-reduce over K=64 clusters).

loss = sum_{k: n_k>1} [ sum_{i in k} ||x_i||^2 - ||s_k||^2 / n_k ],   s_k = sum_{i in k} x_i

Identity used on device (avoids needing per-cluster sum-of-squares):
    loss = T1 - sum_k normsq_k * ( 1[n_k>1]/n_k + 1[n_k==1] )
where T1 = sum_i ||x_i||^2 over ALL rows (a cluster with n_k==1 has
normsq_k == its single row's squared norm; n_k==0 contributes nothing).

Strategy (8 NeuronCores, data-parallel over N):
  - Shard N=200000 rows -> 25000/core, pad to 25088 = 14 chunks x 128 part x 14 rows.
  - Host packs features as bf16 [25088, 257] (col 256 = 1.0 ones column for counts;
    padding rows are all-zero with assignment=64 so they match no cluster).
  - Per chunk: one ~0.92MB DMA; GpSimd builds one-hot [128,14,64] via broadcast
    is_equal(assign, iota); PE accumulates onehot^T @ [x | 1] into PSUM [64,257]
    (cols 0..255 = per-cluster sums, col 256 = per-cluster counts); ACT or DVE
    computes sum(x^2) per partition-chunk (split across both engines to balance).
  - A dummy 256B AllReduce at kernel start absorbs cross-core launch skew and
    ncfw first-call setup while the main loop runs.
  - AllReduce the [64,258] partial (sums, counts, T1) across the 8 cores, finish
    the tiny scalar math identically on each core, output one f32 scalar.
"""

import numpy as np
import ml_dtypes

import concourse.bacc as bacc
import concourse.bass as bass
import concourse.tile as tile
from concourse import mybir
from concourse.bass_utils import run_bass_kernel_spmd

BF16 = mybir.dt.bfloat16
F32 = mybir.dt.float32
P = 128
K = 64            # num clusters
D = 256           # feature dim
MOV = D + 1       # moving columns: features + ones

# full-size problem config
N_TOTAL = 200000
N_CORES = 8
ROWS_REAL = N_TOTAL // N_CORES      # 25000
CHUNK = 14                          # subtiles (matmuls) per DMA chunk
N_CHUNKS = 14
ROWS_PAD = N_CHUNKS * CHUNK * P     # 25088


def default_dve_chunks(n_chunks):
    """Chunks whose square+rowsum runs on VectorE (rest on ScalarE).
    DVE also builds every chunk's one-hot, so it gets fewer squares."""
    if n_chunks < 6:
        return {1} if n_chunks > 1 else set()
    return {n_chunks // 4, n_chunks // 2, (3 * n_chunks) // 4}


def build_nc(n_cores=N_CORES, n_chunks=N_CHUNKS, chunk=CHUNK,
             dve_chunks=None, correction=float(N_TOTAL),
             debug_partial=False, use_ttr=False, onehot_engine="vector",
             dummy_ar=True, bufs=4):
    """Build the SPMD Bass program. Inputs per core:
       feat [rows_pad, MOV] bf16, assign_t [P, n_sub] bf16, iota [P, K] bf16.
       Output: out [1,1] f32 (identical on every core)."""
    n_sub = n_chunks * chunk
    rows_pad = n_sub * P
    block = chunk * P  # rows per chunk
    if dve_chunks is None:
        dve_chunks = default_dve_chunks(n_chunks)

    nc = bacc.Bacc("TRN2", target_bir_lowering=False, debug=False,
                   num_devices=n_cores)

    feat_d = nc.dram_tensor("feat", [rows_pad, MOV], BF16, kind="ExternalInput")
    assign_d = nc.dram_tensor("assign_t", [P, n_sub], BF16, kind="ExternalInput")
    iota_d = nc.dram_tensor("iota", [P, K], BF16, kind="ExternalInput")
    if debug_partial:
        out_d = nc.dram_tensor("out", [K, MOV + 1], F32, kind="ExternalOutput")
    else:
        out_d = nc.dram_tensor("out", [1, 1], F32, kind="ExternalOutput")

    with tile.TileContext(nc) as tc:
        with (
            tc.tile_pool(name="io", bufs=1) as io,
            tc.tile_pool(name="bufp", bufs=bufs) as bufp,
            tc.tile_pool(name="ohp", bufs=bufs) as ohp,
            tc.tile_pool(name="psum", bufs=1, space="PSUM") as psum,
            tc.tile_pool(name="dram", bufs=1, space="DRAM") as dram,
        ):
            rg = [list(range(n_cores))]

            if dummy_ar and not debug_partial:
                dummy_sb = io.tile([K, 1], F32)
                nc.vector.memset(dummy_sb[:], 0.0)
                dummy_in = dram.tile([K, 1], F32)
                dummy_out = dram.tile([K, 1], F32)
                nc.sync.dma_start(out=dummy_in[:], in_=dummy_sb[:])
                nc.gpsimd.collective_compute(
                    "AllReduce", mybir.AluOpType.add, replica_groups=rg,
                    ins=[dummy_in[:].opt()], outs=[dummy_out[:].opt()],
                )

            asg = io.tile([P, n_sub], BF16)
            nc.sync.dma_start(out=asg[:], in_=assign_d[:])
            iot = io.tile([P, K], BF16)
            nc.sync.dma_start(out=iot[:], in_=iota_d[:])

            ones_sb = io.tile([P, 1], F32)
            nc.vector.memset(ones_sb[:], 1.0)
            t1a = io.tile([P, n_chunks], F32)
            nc.vector.memset(t1a[:], 0.0)
            t1d = io.tile([P, n_chunks], F32)
            nc.vector.memset(t1d[:], 0.0)
            scr_a = io.tile([P, chunk, MOV], BF16)
            scr_d = io.tile([P, chunk, MOV], BF16)

            acc = psum.tile([K, MOV], F32, space="PSUM")
            feat_ap = feat_d[:]
            oh_eng = nc.gpsimd if onehot_engine == "gpsimd" else nc.vector

            for s in range(n_chunks):
                buf = bufp.tile([P, chunk, MOV], BF16, name="buf")
                nc.sync.dma_start(
                    out=buf[:],
                    in_=feat_ap[s * block:(s + 1) * block, :].rearrange(
                        "(p n) m -> p n m", n=chunk),
                )
                oh = ohp.tile([P, chunk, K], BF16, name="oh")
                oh_eng.tensor_tensor(
                    out=oh[:],
                    in0=asg[:, s * chunk:(s + 1) * chunk]
                        .unsqueeze(-1).to_broadcast([P, chunk, K]),
                    in1=iot[:].unsqueeze(1).to_broadcast([P, chunk, K]),
                    op=mybir.AluOpType.is_equal,
                )
                if s not in dve_chunks:
                    nc.scalar.activation(
                        out=scr_a[:], in_=buf[:],
                        func=mybir.ActivationFunctionType.Square,
                        accum_out=t1a[:, s:s + 1],
                    )
                elif use_ttr:
                    nc.vector.tensor_tensor_reduce(
                        out=scr_d[:], in0=buf[:], in1=buf[:],
                        scale=1.0, scalar=0.0,
                        op0=mybir.AluOpType.mult, op1=mybir.AluOpType.add,
                        accum_out=t1d[:, s:s + 1],
                    )
                else:
                    nc.vector.tensor_tensor(
                        out=scr_d[:], in0=buf[:], in1=buf[:],
                        op=mybir.AluOpType.mult)
                    nc.vector.reduce_sum(
                        out=t1d[:, s:s + 1], in_=scr_d[:],
                        axis=mybir.AxisListType.XY)
                for j in range(chunk):
                    nc.tensor.matmul(
                        out=acc[:], lhsT=oh[:, j, :], rhs=buf[:, j, :],
                        start=(s == 0 and j == 0),
                        stop=(s == n_chunks - 1 and j == chunk - 1),
                    )

            # per-core T1 partial: reduce [P, n_chunks] cols, then partitions via PE
            t1vec = io.tile([P, 1], F32)
            nc.vector.tensor_tensor(out=t1a[:], in0=t1a[:], in1=t1d[:],
                                    op=mybir.AluOpType.add)
            nc.vector.reduce_sum(out=t1vec[:], in_=t1a[:],
                                 axis=mybir.AxisListType.X)
            t1p = psum.tile([1, 1], F32, space="PSUM")
            nc.tensor.matmul(out=t1p[:], lhsT=t1vec[:], rhs=ones_sb[:],
                             start=True, stop=True)

            # pack [64, 258]: cols 0..255 sums, 256 counts, 257 T1 (row 0 only)
            partial = io.tile([K, MOV + 1], F32)
            nc.vector.memset(partial[:], 0.0)
            nc.scalar.copy(out=partial[:, 0:MOV], in_=acc[:])
            nc.scalar.copy(out=partial[0:1, MOV:MOV + 1], in_=t1p[:])

            if debug_partial:
                nc.sync.dma_start(out=out_d[:], in_=partial[:])
            else:
                ar_in = dram.tile([K, MOV + 1], F32)
                ar_out = dram.tile([K, MOV + 1], F32)
                nc.sync.dma_start(out=ar_in[:], in_=partial[:])
                nc.gpsimd.collective_compute(
                    "AllReduce", mybir.AluOpType.add, replica_groups=rg,
                    ins=[ar_in[:].opt()], outs=[ar_out[:].opt()],
                )
                red = io.tile([K, MOV + 1], F32)
                nc.sync.dma_start(out=red[:], in_=ar_out[:])

                # finisher (identical on all cores)
                scr_f = io.tile([K, D], F32)
                normsq = io.tile([K, 1], F32)
                if use_ttr:
                    nc.vector.tensor_tensor_reduce(
                        out=scr_f[:], in0=red[:, 0:D], in1=red[:, 0:D],
                        scale=1.0, scalar=0.0,
                        op0=mybir.AluOpType.mult, op1=mybir.AluOpType.add,
                        accum_out=normsq[:],
                    )
                else:
                    nc.vector.tensor_tensor(
                        out=scr_f[:], in0=red[:, 0:D], in1=red[:, 0:D],
                        op=mybir.AluOpType.mult)
                    nc.vector.reduce_sum(out=normsq[:], in_=scr_f[:],
                                         axis=mybir.AxisListType.X)
                counts = red[:, D:D + 1]
                safe = io.tile([K, 1], F32)
                nc.vector.tensor_scalar(out=safe[:], in0=counts, scalar1=1.0,
                                        scalar2=None, op0=mybir.AluOpType.max)
                inv = io.tile([K, 1], F32)
                nc.vector.reciprocal(out=inv[:], in_=safe[:])
                maskgt = io.tile([K, 1], F32)
                nc.vector.tensor_scalar(out=maskgt[:], in0=counts, scalar1=1.0,
                                        scalar2=None,
                                        op0=mybir.AluOpType.is_gt)
                maskeq = io.tile([K, 1], F32)
                nc.vector.tensor_scalar(out=maskeq[:], in0=counts, scalar1=1.0,
                                        scalar2=None,
                                        op0=mybir.AluOpType.is_equal)
                w = io.tile([K, 1], F32)
                nc.vector.tensor_tensor(out=w[:], in0=maskgt[:], in1=inv[:],
                                        op=mybir.AluOpType.mult)
                nc.vector.tensor_tensor(out=w[:], in0=w[:], in1=maskeq[:],
                                        op=mybir.AluOpType.add)
                sub = io.tile([K, 1], F32)
                nc.vector.tensor_tensor(out=sub[:], in0=normsq[:], in1=w[:],
                                        op=mybir.AluOpType.mult)
                subp = psum.tile([1, 1], F32, space="PSUM")
                nc.tensor.matmul(out=subp[:], lhsT=sub[:],
                                 rhs=ones_sb[0:K, :], start=True, stop=True)

                tmp = io.tile([1, 1], F32)
                nc.vector.tensor_scalar(out=tmp[:], in0=red[0:1, MOV:MOV + 1],
                                        scalar1=correction, scalar2=None,
                                        op0=mybir.AluOpType.subtract)
                loss_sb = io.tile([1, 1], F32)
                nc.vector.tensor_tensor(out=loss_sb[:], in0=tmp[:],
                                        in1=subp[0:1, 0:1],
                                        op=mybir.AluOpType.subtract)
                nc.sync.dma_start(out=out_d[:], in_=loss_sb[:])

    nc.compile()
    return nc


def prep_inputs(features, cluster_assignments, n_cores=N_CORES,
                n_chunks=N_CHUNKS, chunk=CHUNK):
    """Shard + pack host inputs. Returns in_maps for run_bass_kernel_spmd."""
    bf = ml_dtypes.bfloat16
    n_sub = n_chunks * chunk
    rows_pad = n_sub * P
    n_total = features.shape[0]
    rows_real = n_total // n_cores
    assert rows_real * n_cores == n_total

    feats = np.asarray(features, dtype=np.float32)
    asg = np.asarray(cluster_assignments).astype(np.float32)

    iota = np.broadcast_to(np.arange(K, dtype=np.float32), (P, K)).astype(bf)

    in_maps = []
    for c in range(n_cores):
        fpad = np.zeros((rows_pad, MOV), dtype=np.float32)
        fpad[:rows_real, :D] = feats[c * rows_real:(c + 1) * rows_real]
        fpad[:rows_real, D] = 1.0
        apad = np.full((rows_pad,), float(K), dtype=np.float32)
        apad[:rows_real] = asg[c * rows_real:(c + 1) * rows_real]
        # assign_t[p, s*chunk + j] must be the cluster of feat row s*block + p*chunk + j
        assign_t = (apad.reshape(n_chunks, P, chunk)
                    .transpose(1, 0, 2).reshape(P, n_sub))
        in_maps.append({
            "feat": fpad.astype(bf),
            "assign_t": assign_t.astype(bf),
            "iota": iota,
        })
    return in_maps


_NC_CACHE = {}


def kernel(features, cluster_assignments):
    key = "full"
    if key not in _NC_CACHE:
        _NC_CACHE[key] = build_nc()
    nc = _NC_CACHE[key]
    in_maps = prep_inputs(features, cluster_assignments)
    res = run_bass_kernel_spmd(nc, in_maps, core_ids=list(range(N_CORES)))
    loss = res.results[0]["out"]
    return np.float32(loss.reshape(())).reshape(())


if __name__ == "__main__":
    rng = np.random.default_rng(0)
    f = rng.standard_normal((N_TOTAL, D)).astype(np.float32)
    a = rng.integers(0, K, size=(N_TOTAL,)).astype(np.int64)
    got = kernel(f, a)
    oh = np.zeros((N_TOTAL, K), np.float32)
    oh[np.arange(N_TOTAL), a] = 1.0
    counts = oh.sum(0)
    sums = oh.T @ f
    sumsq = oh.T @ (f * f).sum(1)
    per = sumsq - (sums * sums).sum(1) / np.maximum(counts, 1.0)
    want = per[counts > 1].sum()
    print("got", got, "want", want, "rel", abs(got - want) / abs(want))
